# revision 1
# baseline (speedup 1.0000x reference)
"""FFJORD forward (2 stacked bijectors, RK4, Hutchinson trace) on 8 TRN2 cores.

Data-parallel: batch 4096 split as 512 rows/core, weights replicated.
Feature-major activations ([feature, batch]); every matmul is
lhsT=weight-chunk (bf16/fp8), rhs=activation, N=512.

Structure per aug-eval (64 total, ~32 effective matmul slots each):
  y-chain: z1 (4 MM) -> tanh h1 (bf16) -> z2 (16 MM) -> tanh h2 ->
           z3 column-paired with prev eval's uo via tile_position ->
           in0 = yorig + cc*z3 (DVE, bf16)
  JVP (lagged): d1 = (h1^2-1)*u1 via custom fused DVE op -> fp8,
           u2 = W2^T d1 via fp8 DoubleRow (8 MM, chunk-wise, 2-buffer
           PSUM cycle), d2 = (h2^2-1)*u2 (custom op, per chunk),
           uo = W3^T d2 (col-paired with next z3), mpair = uo*eps,
           l accumulated in a dedicated PSUM bank across all evals.
RK4 yb-bias terms are folded into per-variant tanh-bias tables (b1v),
so in0 updates read yorig directly. fp8 quantization uses 4-phase
antithetic dither on W2/u1 with exact scale compensation folded into
the l-matmul weights (onesw).
"""
import sys

sys.path.insert(0, "/opt/trn_rl_repo")

import numpy as np

B, D, C, H = 4096, 64, 16, 512
NBIJ, NSTEPS = 2, 8
NCORES = 8
BC = B // NCORES          # 512 batch rows per core
NH = H // 128             # 4 hidden chunks
DT = 1.0 / NSTEPS
NPH = 4                   # fp8 dither phases
GW = [0.03, -0.03, 0.015, -0.015]   # W2 dither per phase
GD = [0.02, -0.02, 0.01, -0.01]     # u1 dither per phase

_CACHE = {}
_DVE_OP = {}


def _register_dve_op():
    """Register the fused (sq(in0) - 1) * in1 custom DVE op."""
    if "op" in _DVE_OP:
        return _DVE_OP["op"]
    from concourse import dve_ops
    from concourse.dve_spec import (
        Spec, Src0, Src1, One, sq, lower as dve_lower, _has_src1,
    )
    from concourse.dve_uop import DveOpSpec

    NAME = "SQM1_MUL_ANT"
    if NAME in dve_ops._SUB_OPCODE_FOR_NAME:
        op = next(o for o in dve_ops.OPS if o.name == NAME)
        _DVE_OP["op"] = op
        return op
    spec = Spec(
        body=(sq(Src0) - One) * Src1,
        reference=lambda in0, in1: (in0 * in0 - 1.0) * in1,
    )
    row = dve_ops._CUSTOM_DVE_ROW_BASE + len(dve_ops.OPS)
    dve_ops._SUB_OPCODE_FOR_NAME[NAME] = row
    shas = {
        v: DveOpSpec(
            name=NAME, opcode=row, uops=dve_lower(spec, ver=v),
            rd1_en=_has_src1(spec),
        ).sha(v)
        for v in ("v3", "v4")
    }
    op = dve_ops.DveOp(NAME, spec, subdim=False, uops_sha=shas)
    dve_ops.OPS.append(op)
    dve_ops.CUSTOM_DVE_SPECS[NAME] = spec
    _DVE_OP["op"] = op
    return op


def _build(nbij, nsteps):
    import concourse.bass as bass
    import concourse.tile as tile
    from concourse import bacc, mybir

    SQM1 = _register_dve_op()

    FP32 = mybir.dt.float32
    FP32R = mybir.dt.float32r
    BF16 = mybir.dt.bfloat16
    FP8 = mybir.dt.float8e4
    AF = mybir.ActivationFunctionType
    ALU = mybir.AluOpType
    DR = mybir.MatmulPerfMode.DoubleRow
    ts = bass.ts
    dt = 1.0 / nsteps

    nc = bacc.Bacc(None, target_bir_lowering=False, debug=True)

    # ---- DRAM parameters (per-core views; weights replicated) ----
    xc_d = nc.declare_dram_parameter("xc", [D + C, BC], BF16, isOutput=False)
    x0_d = nc.declare_dram_parameter("x0", [D, BC], FP32, isOutput=False)
    eps_d = nc.declare_dram_parameter("epsT", [nbij, D, BC], FP32R, isOutput=False)
    W1_d = nc.declare_dram_parameter("W1b", [nbij, D + C, H], BF16, isOutput=False)
    W1t_d = nc.declare_dram_parameter("W1t", [nbij, 128, NH], FP32, isOutput=False)
    b1v_d = nc.declare_dram_parameter("b1v", [nbij, 3, 128, NH], FP32, isOutput=False)
    b2_d = nc.declare_dram_parameter("b2c", [nbij, 128, NH], FP32, isOutput=False)
    W2_d = nc.declare_dram_parameter("W2b", [nbij, 128, NH * H], BF16, isOutput=False)
    W28_d = nc.declare_dram_parameter(
        "W28", [nbij * NPH, 128, 2, 2, H], FP8, isOutput=False)
    W3_d = nc.declare_dram_parameter("W3b", [nbij, 128, NH * D], BF16, isOutput=False)
    u1_d = nc.declare_dram_parameter(
        "u1t", [nbij * NPH, 128, NH, BC], BF16, isOutput=False)
    b3dt_d = nc.declare_dram_parameter("b3dt", [nbij, D, 1], FP32, isOutput=False)
    ones_d = nc.declare_dram_parameter("onesw", [2 * D, 2], FP32R, isOutput=False)
    out_d = nc.declare_dram_parameter("out", [D + 1, BC], FP32, isOutput=True)

    with tile.TileContext(nc) as tc:
        with (
            tc.tile_pool(name="const", bufs=1) as const,
            tc.tile_pool(name="h1p", bufs=2) as h1p,
            tc.tile_pool(name="h2p", bufs=2) as h2p,
            tc.tile_pool(name="d1p", bufs=2) as d1p,
            tc.tile_pool(name="d2p", bufs=2) as d2p,
            tc.tile_pool(name="tbp", bufs=3) as tbp,
            tc.tile_pool(name="ksp", bufs=2) as ksp,
            tc.tile_pool(name="mpp", bufs=2) as mpp,
            tc.tile_pool(name="zp", bufs=4, space="PSUM") as zp,
            tc.tile_pool(name="up", bufs=2, space="PSUM") as up,
            tc.tile_pool(name="zop", bufs=1, space="PSUM") as zop,
            tc.tile_pool(name="ltp", bufs=1, space="PSUM") as ltp,
        ):
            # ---- static tiles + loads ----
            in0 = const.tile([D + C, BC], BF16)
            nc.sync.dma_start(in0[:], xc_d[:])
            yorig = const.tile([D, BC], FP32)
            onesw = const.tile([2 * D, 2], FP32R)
            ld_sb = const.tile([1, BC], FP32)
            # warm the tanh table set while DMAs stream
            nc.vector.memset(ld_sb[:], 0.0)
            nc.scalar.activation(ld_sb[0:1, 0:8], ld_sb[0:1, 0:8], AF.Tanh)
            # warm the PE HAM clock gate (cold = 1.2GHz) with dummy
            # matmuls on a zeroed scratch tile during the DMA wait, so
            # real matmuls start at 2.4GHz
            scr = const.tile([128, BC], BF16, name="warmscr")
            nc.vector.memset(scr[:], 0.0)

            W1s, W1t, b1v, b2c, W2s, W3s, epsT = [], [], [], [], [], [], []
            W28, u1t, b3dt = [], [], []
            for ib in range(nbij):
                W1s.append(const.tile([D + C, H], BF16, name=f"w1_{ib}"))
                W1t.append(const.tile([128, NH], FP32, name=f"w1t_{ib}"))
                b1v.append([const.tile([128, NH], FP32, name=f"b1v{v}_{ib}")
                            for v in range(3)])
                b2c.append(const.tile([128, NH], FP32, name=f"b2_{ib}"))
                W2s.append(const.tile([128, NH * H], BF16, name=f"w2_{ib}"))
                W3s.append(const.tile([128, NH * D], BF16, name=f"w3_{ib}"))
                epsT.append(const.tile([D, BC], FP32R, name=f"eps_{ib}"))
                b3dt.append(const.tile([D, 1], FP32, name=f"b3dt_{ib}"))
                W28.append([const.tile([128, 2, 2, H], FP8,
                                       name=f"w28_{ib}_{p}")
                            for p in range(NPH)])
                u1t.append([const.tile([128, NH, BC], BF16,
                                       name=f"u1_{ib}_{p}")
                            for p in range(NPH)])

            # load order: everything eval-0 of bijector 0 touches first
            nc.sync.dma_start(W1s[0][:], W1_d[0])
            nc.sync.dma_start(W1t[0][:], W1t_d[0])
            nc.sync.dma_start(b1v[0][0][:], b1v_d[0, 0])
            nc.sync.dma_start(yorig[:], x0_d[:])
            nc.sync.dma_start(onesw[:], ones_d[:])
            for ib in range(nbij):
                if ib > 0:
                    nc.sync.dma_start(W1s[ib][:], W1_d[ib])
                    nc.sync.dma_start(W1t[ib][:], W1t_d[ib])
                for v in range(3):
                    if ib == 0 and v == 0:
                        continue
                    nc.sync.dma_start(b1v[ib][v][:], b1v_d[ib, v])
                nc.sync.dma_start(W2s[ib][:], W2_d[ib])
                nc.sync.dma_start(b2c[ib][:], b2_d[ib])
                nc.sync.dma_start(W3s[ib][:], W3_d[ib])
                nc.sync.dma_start(b3dt[ib][:], b3dt_d[ib])
                for p in range(NPH):
                    nc.sync.dma_start(u1t[ib][p][:], u1_d[ib * NPH + p])
                    nc.sync.dma_start(W28[ib][p][:], W28_d[ib * NPH + p])
                nc.sync.dma_start(epsT[ib][:], eps_d[ib])

            # PE warmup: ~16 dummy MMs ~= 3.5us of sustained PE activity
            for wi in range(16):
                wz = zp.tile([128, BC], FP32, tag="z")
                nc.tensor.matmul(
                    wz[:], scr[:, 0:128], scr[:],
                    start=True, stop=True,
                )

            # ---- main integration ----
            st = {"prev": None, "mpair": None, "lt": None, "tb": None,
                  "nl": 0}
            total = nbij * nsteps * 4

            def emit_u2_chunk(prev, j):
                """2 DoubleRow MMs: u2 chunk j of prev eval's JVP."""
                u2c = up.tile([128, BC], FP32, tag="u2",
                              name=f"u2_{prev['gi']}_{j}")
                ph = prev["gi"] % NPH
                for p in range(2):
                    nc.tensor.matmul(
                        u2c[:],
                        W28[prev["ib"]][ph][:, p, :, ts(j, 128)],
                        prev["d1"][:, 2 * p:2 * p + 2, :],
                        start=(p == 0), stop=(p == 1),
                        perf_mode=DR,
                    )
                prev["u2"][j] = u2c

            def emit_d2_chunk(prev, j):
                if prev["d2"] is None:
                    prev["d2"] = d2p.tile([128, NH, BC], BF16, tag="d2",
                                          name=f"d2_{prev['gi']}")
                nc.vector._custom_dve(
                    SQM1,
                    out=prev["d2"][:, j, :],
                    in0=prev["h2"][:, j, :],
                    in1=prev["u2"][j][:],
                )

            def emit_jvp_tail(prev, zo):
                """mpair mul; every 2nd eval queues the l-matmul, which is
                emitted later (flush_l) so its late gate can't head-of-line
                block the PE FIFO."""
                gi = prev["gi"]
                if gi % 2 == 0:
                    st["mpair"] = mpp.tile([2 * D, BC], FP32R, tag="mp",
                                           name=f"mp_{gi}")
                half = (gi % 2) * D
                nc.vector.tensor_mul(
                    st["mpair"][half:half + D, :], zo[D:2 * D, :],
                    epsT[prev["ib"]][:].bitcast(FP32),
                )
                if gi % 2 == 1:
                    st["nl"] += 1
                    st["pending_l"] = (st["mpair"], (gi % 4) // 2, st["nl"])

            def flush_l():
                if not st.get("pending_l"):
                    return
                mp, col, nl = st["pending_l"]
                st["pending_l"] = None
                if st["lt"] is None:
                    st["lt"] = ltp.tile([1, BC], FP32, tag="lt", name="lt")
                nc.tensor.matmul(
                    st["lt"][:], onesw[:, col:col + 1], mp[:],
                    start=(nl == 1), stop=(nl == total // 2),
                    skip_group_check=True,
                )

            gi = 0
            for ib in range(nbij):
                for istep in range(nsteps):
                    t0 = istep * dt
                    ksum = ksp.tile([D, BC], FP32, tag="ksum")

                    for e in range(4):
                        t_e = (t0, t0 + dt / 2, t0 + dt / 2, t0 + dt)[e]
                        wgt = (dt / 6, dt / 3, dt / 3, dt / 6)[e]
                        prev = st["prev"]

                        # tanh bias tb = t*W1[80] + b1-variant (v0/vh/vf);
                        # e2 shares e1's (same t, same variant)
                        if e == 2:
                            tb = st["tb"]
                        else:
                            var = (0, 1, 1, 2)[e]
                            tb = tbp.tile([128, NH], FP32, tag="tb")
                            nc.vector.scalar_tensor_tensor(
                                tb[:], W1t[ib][:], float(t_e),
                                b1v[ib][var][:], ALU.mult, ALU.add,
                            )
                        st["tb"] = tb

                        # --- z1 (4 MMs) ---
                        z1s = []
                        for j in range(NH):
                            z1 = zp.tile([128, BC], FP32, tag="z")
                            nc.tensor.matmul(
                                z1[:], W1s[ib][:, ts(j, 128)], in0[:],
                                start=True, stop=True,
                            )
                            z1s.append(z1)
                        # --- h1 tanh (bf16) ---
                        h1t = h1p.tile([128, NH, BC], BF16, tag="h1")
                        for j in range(NH):
                            nc.scalar.activation(
                                h1t[:, j, :], z1s[j][:], AF.Tanh,
                                bias=tb[:, j:j + 1],
                            )

                        # --- u2c2/c3 + d2q2/q3 of prev fill the h1-tanh
                        # window before z2 can start ---
                        if prev:
                            emit_u2_chunk(prev, 2)
                            emit_u2_chunk(prev, 3)
                            emit_d2_chunk(prev, 2)
                            emit_d2_chunk(prev, 3)
                        h2t = h2p.tile([128, NH, BC], BF16, tag="h2")
                        z2s = []
                        for j in range(NH):
                            z2 = zp.tile([128, BC], FP32, tag="z")
                            for kc in range(NH):
                                nc.tensor.matmul(
                                    z2[:],
                                    W2s[ib][:, kc * H + j * 128:
                                            kc * H + (j + 1) * 128],
                                    h1t[:, kc, :],
                                    start=(kc == 0), stop=(kc == NH - 1),
                                )
                            z2s.append(z2)
                            nc.scalar.activation(
                                h2t[:, j, :], z2[:], AF.Tanh,
                                bias=b2c[ib][:, j:j + 1],
                            )
                            if j == 1:
                                # d1 of THIS eval (after h1 tanhs)
                                d1t = d1p.tile([128, NH, BC], FP8, tag="d1")
                                nc.vector._custom_dve(
                                    SQM1, out=d1t[:], in0=h1t[:],
                                    in1=u1t[ib][gi % NPH][:],
                                )
                                if e == 3:
                                    # prefold step-update base off the
                                    # critical path: P = yorig + ksum + dt*b3
                                    Pt = ksp.tile([D, BC], FP32, tag="pfold")
                                    nc.vector.scalar_tensor_tensor(
                                        Pt[:], ksum[:], b3dt[ib][:],
                                        yorig[:], ALU.add, ALU.add,
                                    )

                        # --- z3 | uo(prev) column-paired ---
                        zo = zop.tile([128, BC], FP32, tag="zo")
                        for kc in range(NH):
                            nc.tensor.matmul(
                                zo[0:D, :], W3s[ib][:, ts(kc, D)],
                                h2t[:, kc, :],
                                start=(kc == 0), stop=(kc == NH - 1),
                                tile_position=(0, 0),
                                skip_group_check=True,
                            )
                            if prev:
                                nc.tensor.matmul(
                                    zo[D:2 * D, :],
                                    W3s[prev["ib"]][:, ts(kc, D)],
                                    prev["d2"][:, kc, :],
                                    start=(kc == 0), stop=(kc == NH - 1),
                                    tile_position=(0, 64),
                                    skip_group_check=True,
                                )
                        # deferred l-matmul from the previous pair (its
                        # mpair operand completed long ago)
                        flush_l()

                        # --- RK4 bookkeeping (DVE) — in0 first, it gates
                        # the next eval's z1 ---
                        z3 = zo[0:D, :]
                        cur = {"gi": gi, "ib": ib, "h2": h2t, "d1": d1t,
                               "u2": [None] * NH, "d2": None}
                        if e < 3:
                            cc = (dt / 2, dt / 2, dt)[e]
                            nc.vector.scalar_tensor_tensor(
                                in0[0:D, :], z3, cc, yorig[:],
                                ALU.mult, ALU.add,
                            )
                            if e == 0:
                                nc.vector.tensor_scalar_mul(ksum[:], z3, wgt)
                            else:
                                nc.vector.scalar_tensor_tensor(
                                    ksum[:], z3, wgt, ksum[:],
                                    ALU.mult, ALU.add,
                                )
                        else:
                            # Pt = yorig + ksum(e0..e2) + dt*b3 was folded
                            # early (off critical path); e3's tail is one stt
                            if gi < total - 1:
                                nc.vector.scalar_tensor_tensor(
                                    in0[0:D, :], z3, wgt, Pt[:],
                                    ALU.mult, ALU.add,
                                )
                            nc.vector.scalar_tensor_tensor(
                                yorig[:], z3, wgt, Pt[:],
                                ALU.mult, ALU.add,
                            )

                        # --- u2 chunks 0,1 of THIS eval + d2q0/q1 ---
                        emit_u2_chunk(cur, 0)
                        emit_u2_chunk(cur, 1)
                        emit_d2_chunk(cur, 0)
                        emit_d2_chunk(cur, 1)

                        # --- JVP tail of prev (mpair; l deferred) ---
                        if prev:
                            emit_jvp_tail(prev, zo)

                        st["prev"] = cur
                        gi += 1

            # ---- epilogue: finish the final eval's JVP ----
            prev = st["prev"]
            emit_u2_chunk(prev, 2)
            emit_u2_chunk(prev, 3)
            emit_d2_chunk(prev, 2)
            emit_d2_chunk(prev, 3)
            zo = zop.tile([128, BC], FP32, tag="zo")
            for kc in range(NH):
                nc.tensor.matmul(
                    zo[D:2 * D, :], W3s[prev["ib"]][:, ts(kc, D)],
                    prev["d2"][:, kc, :],
                    start=(kc == 0), stop=(kc == NH - 1),
                    tile_position=(0, 64),
                    skip_group_check=True,
                )
            emit_jvp_tail(prev, zo)
            flush_l()

            # ---- write out ----
            nc.vector.tensor_copy(ld_sb[:], st["lt"][:])
            nc.sync.dma_start(out_d[0:D, :], yorig[:])
            nc.sync.dma_start(out_d[D:D + 1, :], ld_sb[:])

    nc.finalize()
    return nc


def _get_nc(nbij=NBIJ, nsteps=NSTEPS):
    key = (nbij, nsteps)
    if key not in _CACHE:
        _CACHE[key] = _build(nbij, nsteps)
    return _CACHE[key]


def _prep_inputs(x, cond, eps, W1, b1, W2, b2, W3, b3, nbij=NBIJ, nsteps=NSTEPS):
    """Host-side layout prep. Returns per-core in_maps."""
    import ml_dtypes

    f32 = np.float32
    bf16 = ml_dtypes.bfloat16
    f8 = ml_dtypes.float8_e4m3
    x = np.asarray(x, f32)
    cond = np.asarray(cond, f32)
    eps = np.asarray(eps, f32)
    W1 = np.asarray(W1, f32)
    b1 = np.asarray(b1, f32)
    W2 = np.asarray(W2, f32)
    b2 = np.asarray(b2, f32)
    W3 = np.asarray(W3, f32)
    b3 = np.asarray(b3, f32)
    dt = f32(1.0 / nsteps)

    # replicated weight-side arrays
    W1b = np.ascontiguousarray(W1[:nbij, :D + C, :]).astype(bf16)
    W1t = W1[:nbij, D + C, :].reshape(nbij, NH, 128).transpose(0, 2, 1).copy()
    # b1 variants with yb-bias folded in: v0 = b1; vh = b1 + (dt/2) W1y^T b3;
    # vf = b1 + dt W1y^T b3  (all laid out [128, NH])
    w1b3 = np.einsum("ndh,nd->nh", W1[:nbij, :D, :], b3[:nbij])  # [nb, H]
    b1vs = np.stack(
        [b1[:nbij], b1[:nbij] + (dt / 2) * w1b3, b1[:nbij] + dt * w1b3],
        axis=1,
    )  # [nb, 3, H]
    b1vc = (b1vs.reshape(nbij, 3, NH, 128).transpose(0, 1, 3, 2)
            .astype(f32).copy())   # [nb, 3, 128, NH]
    b2c = b2[:nbij].reshape(nbij, NH, 128).transpose(0, 2, 1).copy()
    W2b = (W2[:nbij].reshape(nbij, NH, 128, H).transpose(0, 2, 1, 3)
           .reshape(nbij, 128, NH * H).astype(bf16))
    # DoubleRow weights, 4 dither phases:
    # [nb*ph, r, kpair, i, col] = fp8(W2*(1+GW[ph]))[(2*kpair+i)*128 + r, col]
    W28l = []
    for ibb in range(nbij):
        for p in range(NPH):
            W28l.append(
                (W2[ibb] * (1.0 + GW[p]))
                .reshape(2, 2, 128, H).transpose(2, 0, 1, 3).astype(f8)
            )
    W28 = np.stack(W28l, axis=0)
    W3r = (W3[:nbij].reshape(nbij, NH, 128, D).transpose(0, 2, 1, 3)
           .reshape(nbij, 128, NH * D).astype(bf16))
    b3dt = (b3[:nbij] * dt)[:, :, None].astype(f32).copy()
    # l-matmul weights with fp8 dither compensation: eval phases cycle
    # 0,1,2,3 per step; pair (e0,e1)->col0 (dt/6, dt/3), (e2,e3)->col1.
    s = [(1.0 + GD[p]) * (1.0 + GW[p]) for p in range(NPH)]
    onesw = np.stack(
        [
            np.concatenate([np.full(D, dt / 6 / s[0], f32),
                            np.full(D, dt / 3 / s[1], f32)]),
            np.concatenate([np.full(D, dt / 3 / s[2], f32),
                            np.full(D, dt / 6 / s[3], f32)]),
        ],
        axis=1,
    )
    # u1 = eps @ W1[:D] -> per-core [128, NH, BC] bf16, 4 dither phases
    u1 = np.einsum("nbd,ndh->nbh", eps[:nbij], W1[:nbij, :D, :])  # [nb,B,H]

    shared = {
        "W1b": W1b, "W1t": W1t, "b1v": b1vc, "b2c": b2c,
        "W2b": W2b, "W28": W28, "W3b": W3r, "b3dt": b3dt,
        "onesw": onesw,
    }
    in_maps = []
    for ci in range(NCORES):
        sl = slice(ci * BC, (ci + 1) * BC)
        xT = x[sl].T.copy()                 # [D, BC]
        condT = cond[sl].T.copy()           # [C, BC]
        xc = np.concatenate([xT, condT], axis=0).astype(bf16)  # [D+C, BC]
        epsT = eps[:nbij, sl, :].transpose(0, 2, 1).copy()  # [nb, D, BC]
        # [nb, H, BC] -> [nb, NH, 128, BC] -> [nb, 128, NH, BC], 4 phases
        u1c = (u1[:, sl, :].transpose(0, 2, 1)
               .reshape(nbij, NH, 128, BC).transpose(0, 2, 1, 3))
        u1ph = np.stack(
            [(u1c[ibb] * (1.0 + GD[p])).astype(bf16)
             for ibb in range(nbij) for p in range(NPH)],
            axis=0,
        )  # [nb*ph, 128, NH, BC]
        in_maps.append({"xc": xc, "x0": xT, "epsT": epsT, "u1t": u1ph,
                        **shared})
    return in_maps


def kernel(x, cond, eps, W1, b1, W2, b2, W3, b3):
    from concourse.bass_utils import run_bass_kernel_spmd

    nc = _get_nc()
    in_maps = _prep_inputs(x, cond, eps, W1, b1, W2, b2, W3, b3)
    res = run_bass_kernel_spmd(nc, in_maps, core_ids=list(range(NCORES)))
    outs = []
    for ci in range(NCORES):
        o = res.results[ci]["out"]          # [D+1, BC]
        outs.append(np.ascontiguousarray(o.T))  # [BC, D+1]
    return np.concatenate(outs, axis=0).astype(np.float32)

